# revision 14
# baseline (speedup 1.0000x reference)
"""Trainium2 Bass kernel for nn_CapsuleLayer (dynamic routing capsule layer).

Reference computation (fp32, jax):
    u_hat[b,n,i,d] = sum_k W[n,i,d,k] * x[b,i,k]        B=64 N=32 I=1152 D=16 K=8
    b = 0
    for it in 0..2:
        c = softmax(b, axis=n)
        s[b,n,d] = sum_i c[b,n,i] * u_hat[b,n,i,d]
        v = squash(s)        (elementwise squash quirk)
        if it < 2: b += sum_d u_hat[b,n,i,d] * v[b,n,d]
    out = sigmoid(v[...,None] @ dense_w + dense_b)       [B,N,D,1]

Sharding: data-parallel over batch across 8 NeuronCores (B_local=8).
W is replicated; the host pre-lays it out so the kernel streams it with
contiguous >=1MiB DMAs. Per core, u_hat (18.9 MB) stays resident in SBUF in
layout [p=(i_sub,b), f=(i_blk, d, n)], so HBM traffic is one pass over W +
the x shard.

v3 structure (per core):
  - build: float32r PE matmuls (4x fp32 rate; ~2.4e-4 relative u_hat error,
    measured safe against the 2e-2 gate) of a block-diagonal x stationary
    against W moving tiles; ACT drains PSUM->SBUF; s0 = sum_i u_hat is
    accumulated on the PE (f32r) from the SBUF copies, so the DVE is idle
    during the build and the phase is DMA-bound.
  - routing passes, per 6-block chunk, engines in parallel:
      GpSimd: b-update multiply + pairwise d-reduction tree for sub-chunk 0
      DVE:    same for sub-chunk 1, softmax max (negated) / den reductions,
              rsta build, both e-multiplies (f32r out)
      ACT:    exp with the -max folded in as per-block bias, den reciprocal
      PE:     s accumulation with the softmax normalization folded into the
              per-block stationary: lhsT[p,b] = ones_bd[p,b] * (1/den)[p,blk]
              so c = e/den never materializes (f32r, 4x rate).
    The second pass folds the b-logit accumulation into the reduction tree.
  - precision: u_hat storage and all b-logit arithmetic are fp32 (routing
    has near-tie top-2 gaps <0.01 at logit spreads O(100)); only the PE
    multiply operands run at f32r (~1.2e-4), which perturbs the output far
    below the gate (measured ~1e-3 total).
"""

import numpy as np

import concourse.bacc as bacc
import concourse.mybir as mybir
import concourse.tile as tile
from concourse import bass2jax

B, N, I, D, K = 64, 32, 1152, 16, 8
NCORES = 8
BL = B // NCORES          # 8 local batch
ISUB = 16                 # i's per block
IB = I // ISUB            # 72 i-blocks
ND = D * N                # 512 free elems per i-block  (order (d, n))
WG = 4                    # i-blocks per W DMA chunk (1.05 MiB per dma_start)
TG = 6                    # i-blocks per routing chunk
TV = 3                    # i-blocks per elementwise sub-op
EPS = 1e-7

_FP32 = mybir.dt.float32
_FP32R = mybir.dt.float32r

# Engine assignment knobs (tuned on hw).
GPS_SUBS = 0              # GpSimd co-streaming inflates DVE ops ~25%; keep DVE solo
GPS_SMULT_SUBS = 0        # sub-chunks of the e-multiply on GpSimd
BUILD_F32R = False        # f32r build flips near-tie routing (1.5e-2)
ACT_RECIP = False         # den reciprocal on ACT (blocked: accuracy issues)


def _build_nc():
    nc = bacc.Bacc()

    wdt = _FP32R if BUILD_F32R else _FP32
    w_m = nc.dram_tensor("w_m", [128, IB, ND], wdt, kind="ExternalInput")
    xbd = nc.dram_tensor("xbd", [128, IB, 128], wdt, kind="ExternalInput")
    ones_bd = nc.dram_tensor("ones_bd", [128, BL], _FP32, kind="ExternalInput")
    repl8 = nc.dram_tensor("repl8", [128, 128], _FP32, kind="ExternalInput")
    dwb = nc.dram_tensor("dwb", [BL, 2], _FP32, kind="ExternalInput")
    out_d = nc.dram_tensor("out", [BL, ND], _FP32, kind="ExternalOutput")

    with tile.TileContext(nc) as tc:
        with (
            tc.tile_pool(name="singles", bufs=1) as singles,
            tc.tile_pool(name="small", bufs=1) as small,
            tc.tile_pool(name="psum", bufs=6, space="PSUM") as psum,
            tc.tile_pool(name="pvp", bufs=1, space="PSUM") as pvp,
            tc.tile_pool(name="psacc", bufs=1, space="PSUM") as psacc,
        ):
            # ---- persistent SBUF tensors ----
            u_hat = singles.tile([128, IB, D, N], _FP32)      # 144KB/part
            ones_sb = singles.tile([128, BL], _FP32)
            ones_r = singles.tile([128, BL], _FP32R)
            repl_sb = singles.tile([128, 128], _FP32)
            broute = singles.tile([128, IB, N], _FP32)
            bmax = singles.tile([128, IB], _FP32)
            den = singles.tile([128, IB], _FP32)
            v_pad = singles.tile([128, ND], _FP32)
            dwb_sb = singles.tile([BL, 2], _FP32)
            eps_t = singles.tile([BL, 1], _FP32)
            out_sb = singles.tile([BL, ND], _FP32)

            nc.sync.dma_start(out=ones_sb[:], in_=ones_bd[:])
            nc.vector.tensor_copy(out=ones_r[:], in_=ones_sb[:])
            nc.sync.dma_start(out=repl_sb[:], in_=repl8[:])
            nc.sync.dma_start(out=dwb_sb[:], in_=dwb[:])
            nc.vector.memset(eps_t[:], EPS)
            # moving-operand rows 8.. of v_pad must not be NaN garbage
            nc.vector.memset(v_pad[:], 0.0)

            # ---- phase 1: build u_hat + s0 accumulation (all PE + ACT) ----
            ps0 = psacc.tile([BL, ND], _FP32, tag="s_acc")
            S0G = 18
            with (
                tc.tile_pool(name="wpool", bufs=3) as wpool,
                tc.tile_pool(name="xpool", bufs=3) as xpool,
                tc.tile_pool(name="spool", bufs=1) as spool,
            ):
                s_part = spool.tile([128, IB // S0G, D, N], _FP32)
                for g in range(IB // WG):
                    w_tile = wpool.tile([128, WG, ND], wdt)
                    nc.sync.dma_start(
                        out=w_tile[:], in_=w_m[:, g * WG : (g + 1) * WG, :]
                    )
                    x_tile = xpool.tile([128, WG, 128], wdt)
                    nc.sync.dma_start(
                        out=x_tile[:], in_=xbd[:, g * WG : (g + 1) * WG, :]
                    )
                    for j in range(WG):
                        ib = g * WG + j
                        pu = psum.tile([128, ND], _FP32)
                        nc.tensor.matmul(
                            pu[:],
                            lhsT=x_tile[:, j, :],
                            rhs=w_tile[:, j, :],
                            start=True,
                            stop=True,
                        )
                        dst = u_hat[:, ib].rearrange("p d n -> p (d n)")
                        nc.scalar.copy(out=dst, in_=pu[:])

                # s0 partial sums over i_blk groups (DVE, pipelined with the
                # build by the tile scheduler) + 4 accumulating fp32 matmuls.
                ngrp = IB // S0G
                for g in range(ngrp):
                    nc.vector.tensor_reduce(
                        out=s_part[:, g],
                        in_=u_hat[:, g * S0G : (g + 1) * S0G].rearrange(
                            "p a d n -> p d n a"
                        ),
                        axis=mybir.AxisListType.X,
                        op=mybir.AluOpType.add,
                    )
                for g in range(ngrp):
                    nc.tensor.matmul(
                        ps0[:],
                        lhsT=ones_sb[:],
                        rhs=s_part[:, g].rearrange("p d n -> p (d n)"),
                        start=(g == 0),
                        stop=(g == ngrp - 1),
                    )

            with (
                tc.tile_pool(name="tvpool", bufs=2) as tvpool,
                tc.tile_pool(name="tcpool", bufs=1) as tcpool,
                tc.tile_pool(name="epool", bufs=2) as epool,
                tc.tile_pool(name="rpool", bufs=2) as rpool,
            ):

                def squash(ps, scale, out_tile):
                    """v = squash(scale*s) on [BL, ND]; v -> out_tile rows 0..BL."""
                    t_sq = small.tile([BL, ND], _FP32, tag="t_sq")
                    s_sc = small.tile([BL, ND], _FP32, tag="s_sc")
                    r_ = small.tile([BL, ND], _FP32, tag="r_")
                    nc.scalar.activation(
                        out=t_sq[:], in_=ps[:],
                        func=mybir.ActivationFunctionType.Square, scale=scale,
                    )
                    nc.vector.tensor_scalar_mul(out=s_sc[:], in0=ps[:], scalar1=scale)
                    nc.scalar.activation(
                        out=r_[:], in_=t_sq[:],
                        func=mybir.ActivationFunctionType.Sqrt, bias=eps_t[:],
                    )
                    # r <- (1 + t_sq) * r ; r <- 1/r
                    nc.vector.scalar_tensor_tensor(
                        out=r_[:], in0=t_sq[:], scalar=1.0, in1=r_[:],
                        op0=mybir.AluOpType.add, op1=mybir.AluOpType.mult,
                    )
                    nc.vector.reciprocal(out=r_[:], in_=r_[:])
                    # v = s_sc * t_sq * r
                    nc.vector.tensor_mul(out=s_sc[:], in0=s_sc[:], in1=t_sq[:])
                    nc.vector.tensor_tensor(
                        out=out_tile, in0=s_sc[:], in1=r_[:],
                        op=mybir.AluOpType.mult,
                    )

                def broadcast_v():
                    """v_pad rows [0:BL] -> pv [128,(d,n)] replicated over
                    i_sub; the b-update multiply reads the PSUM tile
                    directly."""
                    pv = pvp.tile([128, ND], _FP32, tag="pv")
                    nc.tensor.matmul(
                        pv[:], lhsT=repl_sb[:], rhs=v_pad[:],
                        start=True, stop=True,
                    )
                    return pv.rearrange("p (d n) -> p d n", n=N)

                def bupd_chunk(vsrc, sl, first):
                    """b-update for a TG chunk: DVE multiply (reads v from
                    PSUM), then the pairwise d-tree on GpSimd — it touches
                    only tmp_v/broute, so it never co-streams u_hat with the
                    DVE. The accumulating pass folds the old logits into the
                    last tree level."""
                    tmp = tvpool.tile([128, TG, D, N], _FP32, tag="tmp_v")
                    nc.vector.tensor_tensor(
                        out=tmp[:],
                        in0=u_hat[:, sl],
                        in1=vsrc[:, None, :, :].to_broadcast([128, TG, D, N]),
                        op=mybir.AluOpType.mult,
                    )
                    for h in (8, 4, 2):
                        nc.gpsimd.tensor_add(
                            out=tmp[:, :, 0:h, :],
                            in0=tmp[:, :, 0:h, :],
                            in1=tmp[:, :, h : 2 * h, :],
                        )
                    if not first:
                        nc.gpsimd.tensor_add(
                            out=tmp[:, :, 0, :],
                            in0=tmp[:, :, 0, :],
                            in1=broute[:, sl],
                        )
                    nc.gpsimd.tensor_add(
                        out=broute[:, sl],
                        in0=tmp[:, :, 0, :],
                        in1=tmp[:, :, 1, :],
                    )

                def fused_pass(first, vsrc):
                    """Per chunk: b-update (iter t) -> softmax (iter t+1) ->
                    e-multiply -> PE accumulation with 1/den in the
                    stationary. Work is spread over DVE/GpSimd/ACT/PE."""
                    ps = psacc.tile([BL, ND], _FP32, tag="s_acc")
                    nmm = 0
                    nsub = TG // TV
                    for g in range(IB // TG):
                        sl = slice(g * TG, (g + 1) * TG)
                        bupd_chunk(vsrc, sl, first)
                        # ---- softmax over n: -max -> exp(x + bias) -> 1/den
                        nc.vector.tensor_reduce(
                            out=bmax[:, sl], in_=broute[:, sl],
                            axis=mybir.AxisListType.X, op=mybir.AluOpType.max,
                            negate=True,
                        )
                        e = epool.tile([128, TG, N], _FP32, tag="e")
                        for j in range(TG):
                            ib = g * TG + j
                            nc.scalar.activation(
                                out=e[:, j], in_=broute[:, ib],
                                func=mybir.ActivationFunctionType.Exp,
                                bias=bmax[:, ib : ib + 1],
                            )
                        nc.vector.tensor_reduce(
                            out=den[:, sl], in_=e[:],
                            axis=mybir.AxisListType.X, op=mybir.AluOpType.add,
                        )
                        if ACT_RECIP:
                            nc.scalar.activation(
                                out=den[:, sl], in_=den[:, sl],
                                func=mybir.ActivationFunctionType.Reciprocal,
                            )
                        else:
                            nc.vector.reciprocal(out=den[:, sl], in_=den[:, sl])
                        # rsta[p,t,b] = ones_bd[p,b] / den[p, g*TG+t] — the
                        # normalization rides the accumulation stationary.
                        rsta = rpool.tile([128, TG, BL], _FP32R, tag="rsta")
                        nc.vector.tensor_tensor(
                            out=rsta[:],
                            in0=ones_sb[:, None, :].to_broadcast([128, TG, BL]),
                            in1=den[:, sl, None].to_broadcast([128, TG, BL]),
                            op=mybir.AluOpType.mult,
                        )
                        # ---- e-multiply + PE accumulation ----
                        tmp_c = tcpool.tile([128, TG, D, N], _FP32R, tag="tmp_c")
                        nc.vector.tensor_tensor(
                            out=tmp_c[:],
                            in0=u_hat[:, sl],
                            in1=e[:, :, None, :].to_broadcast([128, TG, D, N]),
                            op=mybir.AluOpType.mult,
                        )
                        for j in range(TG):
                            nc.tensor.matmul(
                                ps[:],
                                lhsT=rsta[:, j],
                                rhs=tmp_c[:, j].rearrange("p d n -> p (d n)"),
                                start=(nmm == 0),
                                stop=(nmm == IB - 1),
                            )
                            nmm += 1
                    return ps

                # ---- iteration 0: c uniform -> s0 = sum_i u_hat / N ----
                squash(ps0, 1.0 / N, v_pad[:BL, :])

                # ---- b-update 0 + softmax 1 + s1 ----
                pv0 = broadcast_v()
                ps1 = fused_pass(first=True, vsrc=pv0)
                squash(ps1, 1.0, v_pad[:BL, :])
                pv1 = broadcast_v()

                # ---- b-update 1 + softmax 2 + s2 ----
                ps2 = fused_pass(first=False, vsrc=pv1)
                v2 = small.tile([BL, ND], _FP32, tag="v2")
                squash(ps2, 1.0, v2[:])
                # out = sigmoid(dense_w * v2 + dense_b)
                nc.scalar.activation(
                    out=out_sb[:], in_=v2[:],
                    func=mybir.ActivationFunctionType.Sigmoid,
                    scale=dwb_sb[:, 0:1], bias=dwb_sb[:, 1:2],
                )
                nc.sync.dma_start(out=out_d[:], in_=out_sb[:])

    if not nc.is_finalized():
        nc.finalize()
    return nc


_NC_CACHE = None


def _get_nc():
    global _NC_CACHE
    if _NC_CACHE is None:
        _NC_CACHE = _build_nc()
    return _NC_CACHE


class _Runner:
    """Sharded PJRT executor for the SPMD bass program.

    Mirrors bass2jax.run_bass_via_pjrt's multi-core path, but keeps the
    (non-donated) inputs device-resident so repeated calls measure close to
    pure device execution.
    """

    def __init__(self, nc):
        import jax
        from jax.experimental.shard_map import shard_map
        from jax.sharding import Mesh, PartitionSpec

        bass2jax.install_neuronx_cc_hook()
        self.nc = nc
        partition_name = (
            nc.partition_id_tensor.name if nc.partition_id_tensor else None
        )
        in_names, out_names, out_avals, zero_outs = [], [], [], []
        for alloc in nc.m.functions[0].allocations:
            if not isinstance(alloc, mybir.MemoryLocationSet):
                continue
            name = alloc.memorylocations[0].name
            if alloc.kind == "ExternalInput":
                if name != partition_name:
                    in_names.append(name)
            elif alloc.kind == "ExternalOutput":
                shape = tuple(alloc.tensor_shape)
                dtype = mybir.dt.np(alloc.dtype)
                out_names.append(name)
                out_avals.append(jax.core.ShapedArray(shape, dtype))
                zero_outs.append(np.zeros(shape, dtype))
        self.in_names = list(in_names)
        self.out_names = out_names
        self.out_avals = out_avals
        self.zero_outs = zero_outs
        n_params = len(in_names)
        n_outs = len(out_avals)
        all_in = in_names + out_names
        donate = tuple(range(n_params, n_params + n_outs))

        def _body(*args):
            operands = list(args)
            if partition_name is not None:
                operands.append(bass2jax.partition_id_tensor())
            outs = bass2jax._bass_exec_p.bind(
                *operands,
                out_avals=tuple(out_avals),
                in_names=tuple(all_in + ([partition_name] if partition_name else [])),
                out_names=tuple(out_names),
                lowering_input_output_aliases=(),
                sim_require_finite=True,
                sim_require_nnan=True,
                nc=nc,
            )
            return tuple(outs)

        devices = jax.devices()[:NCORES]
        self.mesh = Mesh(np.asarray(devices), ("core",))
        in_specs = (PartitionSpec("core"),) * (n_params + n_outs)
        out_specs = (PartitionSpec("core"),) * n_outs
        self.sharded = jax.jit(
            shard_map(
                _body, mesh=self.mesh, in_specs=in_specs,
                out_specs=out_specs, check_rep=False,
            ),
            donate_argnums=donate,
            keep_unused=True,
        )
        self._jax = jax
        self._pspec = PartitionSpec

    def place_inputs(self, in_maps):
        import jax

        sharding = jax.sharding.NamedSharding(self.mesh, self._pspec("core"))
        concat_in = [
            np.concatenate([m[name] for m in in_maps], axis=0)
            for name in self.in_names
        ]
        self.dev_in = [jax.device_put(a, sharding) for a in concat_in]

    def run(self):
        zeros = [
            np.zeros((NCORES * z.shape[0], *z.shape[1:]), z.dtype)
            for z in self.zero_outs
        ]
        out_arrs = self.sharded(*self.dev_in, *zeros)
        self._jax.block_until_ready(out_arrs)
        return out_arrs

    def results(self, out_arrs):
        return [
            {
                name: np.asarray(out_arrs[i]).reshape(
                    NCORES, *self.out_avals[i].shape
                )[c]
                for i, name in enumerate(self.out_names)
            }
            for c in range(NCORES)
        ]


_RUNNER_CACHE = None


def _get_runner():
    global _RUNNER_CACHE
    if _RUNNER_CACHE is None:
        _RUNNER_CACHE = _Runner(_get_nc())
    return _RUNNER_CACHE


def _host_prep(x, W):
    """Build the host-side input arrays for each core."""
    # W moving layout: w_m[p=(i_sub,k), ib, (d,n)] = W[n, ib*16+i_sub, d, k]
    w_m = W.reshape(N, IB, ISUB, D, K)                           # n ib isub d k
    w_m = w_m.transpose(2, 4, 1, 3, 0)                           # isub k ib d n
    w_m = np.ascontiguousarray(w_m.reshape(128, IB, ND), dtype=np.float32)

    ones_bd = np.zeros((128, BL), dtype=np.float32)
    for isub in range(ISUB):
        for b_ in range(BL):
            ones_bd[isub * BL + b_, b_] = 1.0
    repl8 = np.zeros((128, 128), dtype=np.float32)
    for b_ in range(BL):
        repl8[b_, b_::BL] = 1.0

    shards = []
    for c in range(NCORES):
        xc = x[c * BL : (c + 1) * BL]                            # [BL, I, K]
        # xbd[p=(i_sub,k), ib, q=(i_sub',b)] block-diagonal in i_sub
        xbd = np.zeros((128, IB, 128), dtype=np.float32)
        xcr = np.ascontiguousarray(
            xc.reshape(BL, IB, ISUB, K).transpose(2, 3, 1, 0)
        )  # [isub, k, ib, b]
        for isub in range(ISUB):
            xbd[
                isub * K : (isub + 1) * K, :, isub * BL : (isub + 1) * BL
            ] = xcr[isub]
        shards.append(xbd)
    return w_m, ones_bd, repl8, shards


def _prepare_in_maps(x, W, dense_w, dense_b):
    w_m, ones_bd, repl8, xbds = _host_prep(x, W)
    dwb = np.tile(
        np.array([[dense_w[0, 0], dense_b[0]]], dtype=np.float32), (BL, 1)
    )
    return [
        {"w_m": w_m, "xbd": xbds[c], "ones_bd": ones_bd, "repl8": repl8,
         "dwb": dwb}
        for c in range(NCORES)
    ]


def _gather_core_output(o):
    """[BL, ND] (d,n order) -> [BL, N, D, 1]."""
    o = np.asarray(o).reshape(BL, D, N).transpose(0, 2, 1)
    return o[..., None].astype(np.float32)


def _gather_output(results):
    outs = [_gather_core_output(results[c]["out"]) for c in range(NCORES)]
    return np.concatenate(outs, axis=0)


def kernel(x, W, dense_w, dense_b):
    x = np.asarray(x, dtype=np.float32)
    W = np.asarray(W, dtype=np.float32)
    dense_w = np.asarray(dense_w, dtype=np.float32)
    dense_b = np.asarray(dense_b, dtype=np.float32)

    runner = _get_runner()
    runner.place_inputs(_prepare_in_maps(x, W, dense_w, dense_b))
    return _gather_output(runner.results(runner.run()))


def bench(x, W, dense_w, dense_b, repeat=10):
    """Return (output, min wall seconds per run with device-resident inputs)."""
    import time

    x = np.asarray(x, dtype=np.float32)
    W = np.asarray(W, dtype=np.float32)
    runner = _get_runner()
    runner.place_inputs(
        _prepare_in_maps(
            x, W,
            np.asarray(dense_w, dtype=np.float32),
            np.asarray(dense_b, dtype=np.float32),
        )
    )
    out_arrs = runner.run()  # warmup/compile
    times = []
    for _ in range(repeat):
        t0 = time.perf_counter()
        out_arrs = runner.run()
        times.append(time.perf_counter() - t0)
    return _gather_output(runner.results(out_arrs)), min(times)


if __name__ == "__main__":
    nc = _get_nc()
    print("built ok")


# revision 17
# speedup vs baseline: 1.1643x; 1.1643x over previous
"""Trainium2 Bass kernel for nn_CapsuleLayer (dynamic routing capsule layer).

Reference computation (fp32, jax):
    u_hat[b,n,i,d] = sum_k W[n,i,d,k] * x[b,i,k]        B=64 N=32 I=1152 D=16 K=8
    b = 0
    for it in 0..2:
        c = softmax(b, axis=n)
        s[b,n,d] = sum_i c[b,n,i] * u_hat[b,n,i,d]
        v = squash(s)        (elementwise squash quirk)
        if it < 2: b += sum_d u_hat[b,n,i,d] * v[b,n,d]
    out = sigmoid(v[...,None] @ dense_w + dense_b)       [B,N,D,1]

Sharding: data-parallel over batch across 8 NeuronCores (B_local=8).
W is replicated; the host pre-lays it out so the kernel streams it with
contiguous >=1MiB DMAs. Per core, u_hat (18.9 MB) stays resident in SBUF in
layout [p=(i_sub,b), f=(i_blk, d, n)], so HBM traffic is one pass over W +
the x shard.

v3 structure (per core):
  - build: float32r PE matmuls (4x fp32 rate; ~2.4e-4 relative u_hat error,
    measured safe against the 2e-2 gate) of a block-diagonal x stationary
    against W moving tiles; ACT drains PSUM->SBUF; s0 = sum_i u_hat is
    accumulated on the PE (f32r) from the SBUF copies, so the DVE is idle
    during the build and the phase is DMA-bound.
  - routing passes, per 6-block chunk, engines in parallel:
      GpSimd: b-update multiply + pairwise d-reduction tree for sub-chunk 0
      DVE:    same for sub-chunk 1, softmax max (negated) / den reductions,
              rsta build, both e-multiplies (f32r out)
      ACT:    exp with the -max folded in as per-block bias, den reciprocal
      PE:     s accumulation with the softmax normalization folded into the
              per-block stationary: lhsT[p,b] = ones_bd[p,b] * (1/den)[p,blk]
              so c = e/den never materializes (f32r, 4x rate).
    The second pass folds the b-logit accumulation into the reduction tree.
  - precision: u_hat storage and all b-logit arithmetic are fp32 (routing
    has near-tie top-2 gaps <0.01 at logit spreads O(100)); only the PE
    multiply operands run at f32r (~1.2e-4), which perturbs the output far
    below the gate (measured ~1e-3 total).
"""

import numpy as np

import concourse.bacc as bacc
import concourse.mybir as mybir
import concourse.tile as tile
from concourse import bass2jax

B, N, I, D, K = 64, 32, 1152, 16, 8
NCORES = 8
BL = B // NCORES          # 8 local batch
ISUB = 16                 # i's per block
IB = I // ISUB            # 72 i-blocks
ND = D * N                # 512 free elems per i-block  (order (d, n))
WG = 4                    # i-blocks per W DMA chunk (1.05 MiB per dma_start)
TG = 8                    # i-blocks per routing chunk (9 chunks/pass)
TV = 3                    # i-blocks per elementwise sub-op
EPS = 1e-7

_FP32 = mybir.dt.float32
_FP32R = mybir.dt.float32r

# Engine assignment knobs (tuned on hw).
GPS_SUBS = 0              # GpSimd co-streaming inflates DVE ops ~25%; keep DVE solo
GPS_SMULT_SUBS = 0        # sub-chunks of the e-multiply on GpSimd
BUILD_F32R = False        # f32r build flips near-tie routing (1.5e-2)
ACT_RECIP = False         # den reciprocal on ACT (blocked: accuracy issues)


def _build_nc():
    nc = bacc.Bacc()

    wdt = _FP32R if BUILD_F32R else _FP32
    w_m = nc.dram_tensor("w_m", [128, IB, ND], wdt, kind="ExternalInput")
    xbd = nc.dram_tensor("xbd", [128, IB, 128], wdt, kind="ExternalInput")
    ones_bd = nc.dram_tensor("ones_bd", [128, BL], _FP32, kind="ExternalInput")
    repl8 = nc.dram_tensor("repl8", [128, 128], _FP32, kind="ExternalInput")
    dwb = nc.dram_tensor("dwb", [BL, 2], _FP32, kind="ExternalInput")
    out_d = nc.dram_tensor("out", [BL, ND], _FP32, kind="ExternalOutput")

    with tile.TileContext(nc) as tc:
        with (
            tc.tile_pool(name="singles", bufs=1) as singles,
            tc.tile_pool(name="small", bufs=1) as small,
            tc.tile_pool(name="psum", bufs=6, space="PSUM") as psum,
            tc.tile_pool(name="pvp", bufs=1, space="PSUM") as pvp,
            tc.tile_pool(name="psacc", bufs=1, space="PSUM") as psacc,
        ):
            # ---- persistent SBUF tensors ----
            u_hat = singles.tile([128, IB, D, N], _FP32)      # 144KB/part
            ones_sb = singles.tile([128, BL], _FP32)
            ones_r = singles.tile([128, BL], _FP32R)
            repl_sb = singles.tile([128, 128], _FP32)
            broute = singles.tile([128, IB, N], _FP32)
            bmax = singles.tile([128, IB], _FP32)
            den = singles.tile([128, IB], _FP32)
            v_pad = singles.tile([128, ND], _FP32)
            v_bc = singles.tile([128, D, N], _FP32)
            dwb_sb = singles.tile([BL, 2], _FP32)
            eps_t = singles.tile([BL, 1], _FP32)
            out_sb = singles.tile([BL, ND], _FP32)

            nc.sync.dma_start(out=ones_sb[:], in_=ones_bd[:])
            nc.vector.tensor_copy(out=ones_r[:], in_=ones_sb[:])
            nc.sync.dma_start(out=repl_sb[:], in_=repl8[:])
            nc.sync.dma_start(out=dwb_sb[:], in_=dwb[:])
            nc.vector.memset(eps_t[:], EPS)
            # moving-operand rows 8.. of v_pad must not be NaN garbage
            nc.vector.memset(v_pad[:], 0.0)

            # ---- phase 1: build u_hat + s0 accumulation (all PE + ACT) ----
            ps0 = psacc.tile([BL, ND], _FP32, tag="s_acc")
            S0G = 18
            with (
                tc.tile_pool(name="wpool", bufs=3) as wpool,
                tc.tile_pool(name="xpool", bufs=3) as xpool,
                tc.tile_pool(name="spool", bufs=1) as spool,
            ):
                s_part = spool.tile([128, IB // S0G, D, N], _FP32)
                for g in range(IB // WG):
                    w_tile = wpool.tile([128, WG, ND], wdt)
                    nc.sync.dma_start(
                        out=w_tile[:], in_=w_m[:, g * WG : (g + 1) * WG, :]
                    )
                    x_tile = xpool.tile([128, WG, 128], wdt)
                    nc.sync.dma_start(
                        out=x_tile[:], in_=xbd[:, g * WG : (g + 1) * WG, :]
                    )
                    for j in range(WG):
                        ib = g * WG + j
                        pu = psum.tile([128, ND], _FP32)
                        nc.tensor.matmul(
                            pu[:],
                            lhsT=x_tile[:, j, :],
                            rhs=w_tile[:, j, :],
                            start=True,
                            stop=True,
                        )
                        dst = u_hat[:, ib].rearrange("p d n -> p (d n)")
                        nc.scalar.copy(out=dst, in_=pu[:])

                # s0 partial sums over i_blk groups (DVE, pipelined with the
                # build by the tile scheduler) + 4 accumulating fp32 matmuls.
                ngrp = IB // S0G
                for g in range(ngrp):
                    nc.vector.tensor_reduce(
                        out=s_part[:, g],
                        in_=u_hat[:, g * S0G : (g + 1) * S0G].rearrange(
                            "p a d n -> p d n a"
                        ),
                        axis=mybir.AxisListType.X,
                        op=mybir.AluOpType.add,
                    )
                for g in range(ngrp):
                    nc.tensor.matmul(
                        ps0[:],
                        lhsT=ones_sb[:],
                        rhs=s_part[:, g].rearrange("p d n -> p (d n)"),
                        start=(g == 0),
                        stop=(g == ngrp - 1),
                    )

            with (
                tc.tile_pool(name="tvpool", bufs=1) as tvpool,
                tc.tile_pool(name="tcpool", bufs=1) as tcpool,
                tc.tile_pool(name="epool", bufs=2) as epool,
                tc.tile_pool(name="rpool", bufs=2) as rpool,
            ):

                def squash(ps, scale, out_tile):
                    """v = squash(scale*s) on [BL, ND]; v -> out_tile rows 0..BL."""
                    t_sq = small.tile([BL, ND], _FP32, tag="t_sq")
                    s_sc = small.tile([BL, ND], _FP32, tag="s_sc")
                    r_ = small.tile([BL, ND], _FP32, tag="r_")
                    nc.scalar.activation(
                        out=t_sq[:], in_=ps[:],
                        func=mybir.ActivationFunctionType.Square, scale=scale,
                    )
                    nc.vector.tensor_scalar_mul(out=s_sc[:], in0=ps[:], scalar1=scale)
                    nc.scalar.activation(
                        out=r_[:], in_=t_sq[:],
                        func=mybir.ActivationFunctionType.Sqrt, bias=eps_t[:],
                    )
                    # r <- (1 + t_sq) * r ; r <- 1/r
                    nc.vector.scalar_tensor_tensor(
                        out=r_[:], in0=t_sq[:], scalar=1.0, in1=r_[:],
                        op0=mybir.AluOpType.add, op1=mybir.AluOpType.mult,
                    )
                    nc.vector.reciprocal(out=r_[:], in_=r_[:])
                    # v = s_sc * t_sq * r
                    nc.vector.tensor_mul(out=s_sc[:], in0=s_sc[:], in1=t_sq[:])
                    nc.vector.tensor_tensor(
                        out=out_tile, in0=s_sc[:], in1=r_[:],
                        op=mybir.AluOpType.mult,
                    )

                def broadcast_v():
                    """v_pad rows [0:BL] -> v_bc [128,(d,n)] replicated over
                    i_sub (PE replication matmul + ACT copy to SBUF so GpSimd
                    — which cannot read PSUM — can consume it)."""
                    pv = pvp.tile([128, ND], _FP32, tag="pv")
                    nc.tensor.matmul(
                        pv[:], lhsT=repl_sb[:], rhs=v_pad[:],
                        start=True, stop=True,
                    )
                    nc.scalar.copy(
                        out=v_bc[:].rearrange("p d n -> p (d n)"), in_=pv[:]
                    )
                    return v_bc

                def bupd_chunk(eng, sl, first):
                    """b-update for a TG chunk: multiply + pairwise d-tree.
                    For the accumulating pass, the old logits are folded into
                    the last tree level."""
                    tmp = tvpool.tile([128, TG, D, N], _FP32, tag="tmp_v")
                    eng.tensor_tensor(
                        out=tmp[:],
                        in0=u_hat[:, sl],
                        in1=v_bc[:, None, :, :].to_broadcast([128, TG, D, N]),
                        op=mybir.AluOpType.mult,
                    )
                    for h in (8, 4, 2):
                        eng.tensor_add(
                            out=tmp[:, :, 0:h, :],
                            in0=tmp[:, :, 0:h, :],
                            in1=tmp[:, :, h : 2 * h, :],
                        )
                    if not first:
                        eng.tensor_add(
                            out=tmp[:, :, 0, :],
                            in0=tmp[:, :, 0, :],
                            in1=broute[:, sl],
                        )
                    eng.tensor_add(
                        out=broute[:, sl],
                        in0=tmp[:, :, 0, :],
                        in1=tmp[:, :, 1, :],
                    )

                def fused_pass(first, vsrc):
                    """Per chunk: b-update (iter t) -> softmax (iter t+1) ->
                    e-multiply -> PE accumulation with 1/den in the
                    stationary. Work is spread over DVE/GpSimd/ACT/PE."""
                    ps = psacc.tile([BL, ND], _FP32, tag="s_acc")
                    nmm = 0
                    nsub = TG // TV
                    for g in range(IB // TG):
                        sl = slice(g * TG, (g + 1) * TG)
                        bupd_chunk(nc.vector, sl, first)
                        # ---- softmax over n: -max -> exp(x + bias) -> 1/den
                        nc.vector.tensor_reduce(
                            out=bmax[:, sl], in_=broute[:, sl],
                            axis=mybir.AxisListType.X, op=mybir.AluOpType.max,
                            negate=True,
                        )
                        e = epool.tile([128, TG, N], _FP32, tag="e")
                        for j in range(TG):
                            ib = g * TG + j
                            nc.scalar.activation(
                                out=e[:, j], in_=broute[:, ib],
                                func=mybir.ActivationFunctionType.Exp,
                                bias=bmax[:, ib : ib + 1],
                            )
                        nc.vector.tensor_reduce(
                            out=den[:, sl], in_=e[:],
                            axis=mybir.AxisListType.X, op=mybir.AluOpType.add,
                        )
                        if ACT_RECIP:
                            nc.scalar.activation(
                                out=den[:, sl], in_=den[:, sl],
                                func=mybir.ActivationFunctionType.Reciprocal,
                            )
                        else:
                            nc.vector.reciprocal(out=den[:, sl], in_=den[:, sl])
                        # rsta[p,t,b] = ones_bd[p,b] / den[p, g*TG+t] — the
                        # normalization rides the accumulation stationary.
                        rsta = rpool.tile([128, TG, BL], _FP32R, tag="rsta")
                        nc.vector.tensor_tensor(
                            out=rsta[:],
                            in0=ones_sb[:, None, :].to_broadcast([128, TG, BL]),
                            in1=den[:, sl, None].to_broadcast([128, TG, BL]),
                            op=mybir.AluOpType.mult,
                        )
                        # ---- e-multiply + PE accumulation ----
                        tmp_c = tcpool.tile([128, TG, D, N], _FP32R, tag="tmp_c")
                        nc.vector.tensor_tensor(
                            out=tmp_c[:],
                            in0=u_hat[:, sl],
                            in1=e[:, :, None, :].to_broadcast([128, TG, D, N]),
                            op=mybir.AluOpType.mult,
                        )
                        for j in range(TG):
                            nc.tensor.matmul(
                                ps[:],
                                lhsT=rsta[:, j],
                                rhs=tmp_c[:, j].rearrange("p d n -> p (d n)"),
                                start=(nmm == 0),
                                stop=(nmm == IB - 1),
                            )
                            nmm += 1
                    return ps

                # ---- iteration 0: c uniform -> s0 = sum_i u_hat / N ----
                squash(ps0, 1.0 / N, v_pad[:BL, :])
                broadcast_v()

                # ---- b-update 0 + softmax 1 + s1 ----
                ps1 = fused_pass(first=True, vsrc=v_bc)
                squash(ps1, 1.0, v_pad[:BL, :])
                broadcast_v()

                # ---- b-update 1 + softmax 2 + s2 ----
                ps2 = fused_pass(first=False, vsrc=v_bc)
                v2 = small.tile([BL, ND], _FP32, tag="v2")
                squash(ps2, 1.0, v2[:])
                # out = sigmoid(dense_w * v2 + dense_b)
                nc.scalar.activation(
                    out=out_sb[:], in_=v2[:],
                    func=mybir.ActivationFunctionType.Sigmoid,
                    scale=dwb_sb[:, 0:1], bias=dwb_sb[:, 1:2],
                )
                nc.sync.dma_start(out=out_d[:], in_=out_sb[:])

    if not nc.is_finalized():
        nc.finalize()
    return nc


_NC_CACHE = None


def _get_nc():
    global _NC_CACHE
    if _NC_CACHE is None:
        _NC_CACHE = _build_nc()
    return _NC_CACHE


class _Runner:
    """Sharded PJRT executor for the SPMD bass program.

    Mirrors bass2jax.run_bass_via_pjrt's multi-core path, but keeps the
    (non-donated) inputs device-resident so repeated calls measure close to
    pure device execution.
    """

    def __init__(self, nc):
        import jax
        from jax.experimental.shard_map import shard_map
        from jax.sharding import Mesh, PartitionSpec

        bass2jax.install_neuronx_cc_hook()
        self.nc = nc
        partition_name = (
            nc.partition_id_tensor.name if nc.partition_id_tensor else None
        )
        in_names, out_names, out_avals, zero_outs = [], [], [], []
        for alloc in nc.m.functions[0].allocations:
            if not isinstance(alloc, mybir.MemoryLocationSet):
                continue
            name = alloc.memorylocations[0].name
            if alloc.kind == "ExternalInput":
                if name != partition_name:
                    in_names.append(name)
            elif alloc.kind == "ExternalOutput":
                shape = tuple(alloc.tensor_shape)
                dtype = mybir.dt.np(alloc.dtype)
                out_names.append(name)
                out_avals.append(jax.core.ShapedArray(shape, dtype))
                zero_outs.append(np.zeros(shape, dtype))
        self.in_names = list(in_names)
        self.out_names = out_names
        self.out_avals = out_avals
        self.zero_outs = zero_outs
        n_params = len(in_names)
        n_outs = len(out_avals)
        all_in = in_names + out_names
        donate = tuple(range(n_params, n_params + n_outs))

        def _body(*args):
            operands = list(args)
            if partition_name is not None:
                operands.append(bass2jax.partition_id_tensor())
            outs = bass2jax._bass_exec_p.bind(
                *operands,
                out_avals=tuple(out_avals),
                in_names=tuple(all_in + ([partition_name] if partition_name else [])),
                out_names=tuple(out_names),
                lowering_input_output_aliases=(),
                sim_require_finite=True,
                sim_require_nnan=True,
                nc=nc,
            )
            return tuple(outs)

        devices = jax.devices()[:NCORES]
        self.mesh = Mesh(np.asarray(devices), ("core",))
        in_specs = (PartitionSpec("core"),) * (n_params + n_outs)
        out_specs = (PartitionSpec("core"),) * n_outs
        self.sharded = jax.jit(
            shard_map(
                _body, mesh=self.mesh, in_specs=in_specs,
                out_specs=out_specs, check_rep=False,
            ),
            donate_argnums=donate,
            keep_unused=True,
        )
        self._jax = jax
        self._pspec = PartitionSpec

    def place_inputs(self, in_maps):
        import jax

        sharding = jax.sharding.NamedSharding(self.mesh, self._pspec("core"))
        concat_in = [
            np.concatenate([m[name] for m in in_maps], axis=0)
            for name in self.in_names
        ]
        self.dev_in = [jax.device_put(a, sharding) for a in concat_in]

    def run(self):
        zeros = [
            np.zeros((NCORES * z.shape[0], *z.shape[1:]), z.dtype)
            for z in self.zero_outs
        ]
        out_arrs = self.sharded(*self.dev_in, *zeros)
        self._jax.block_until_ready(out_arrs)
        return out_arrs

    def results(self, out_arrs):
        return [
            {
                name: np.asarray(out_arrs[i]).reshape(
                    NCORES, *self.out_avals[i].shape
                )[c]
                for i, name in enumerate(self.out_names)
            }
            for c in range(NCORES)
        ]


_RUNNER_CACHE = None


def _get_runner():
    global _RUNNER_CACHE
    if _RUNNER_CACHE is None:
        _RUNNER_CACHE = _Runner(_get_nc())
    return _RUNNER_CACHE


def _host_prep(x, W):
    """Build the host-side input arrays for each core."""
    # W moving layout: w_m[p=(i_sub,k), ib, (d,n)] = W[n, ib*16+i_sub, d, k]
    w_m = W.reshape(N, IB, ISUB, D, K)                           # n ib isub d k
    w_m = w_m.transpose(2, 4, 1, 3, 0)                           # isub k ib d n
    w_m = np.ascontiguousarray(w_m.reshape(128, IB, ND), dtype=np.float32)

    ones_bd = np.zeros((128, BL), dtype=np.float32)
    for isub in range(ISUB):
        for b_ in range(BL):
            ones_bd[isub * BL + b_, b_] = 1.0
    repl8 = np.zeros((128, 128), dtype=np.float32)
    for b_ in range(BL):
        repl8[b_, b_::BL] = 1.0

    shards = []
    for c in range(NCORES):
        xc = x[c * BL : (c + 1) * BL]                            # [BL, I, K]
        # xbd[p=(i_sub,k), ib, q=(i_sub',b)] block-diagonal in i_sub
        xbd = np.zeros((128, IB, 128), dtype=np.float32)
        xcr = np.ascontiguousarray(
            xc.reshape(BL, IB, ISUB, K).transpose(2, 3, 1, 0)
        )  # [isub, k, ib, b]
        for isub in range(ISUB):
            xbd[
                isub * K : (isub + 1) * K, :, isub * BL : (isub + 1) * BL
            ] = xcr[isub]
        shards.append(xbd)
    return w_m, ones_bd, repl8, shards


def _prepare_in_maps(x, W, dense_w, dense_b):
    w_m, ones_bd, repl8, xbds = _host_prep(x, W)
    dwb = np.tile(
        np.array([[dense_w[0, 0], dense_b[0]]], dtype=np.float32), (BL, 1)
    )
    return [
        {"w_m": w_m, "xbd": xbds[c], "ones_bd": ones_bd, "repl8": repl8,
         "dwb": dwb}
        for c in range(NCORES)
    ]


def _gather_core_output(o):
    """[BL, ND] (d,n order) -> [BL, N, D, 1]."""
    o = np.asarray(o).reshape(BL, D, N).transpose(0, 2, 1)
    return o[..., None].astype(np.float32)


def _gather_output(results):
    outs = [_gather_core_output(results[c]["out"]) for c in range(NCORES)]
    return np.concatenate(outs, axis=0)


def kernel(x, W, dense_w, dense_b):
    x = np.asarray(x, dtype=np.float32)
    W = np.asarray(W, dtype=np.float32)
    dense_w = np.asarray(dense_w, dtype=np.float32)
    dense_b = np.asarray(dense_b, dtype=np.float32)

    runner = _get_runner()
    runner.place_inputs(_prepare_in_maps(x, W, dense_w, dense_b))
    return _gather_output(runner.results(runner.run()))


def bench(x, W, dense_w, dense_b, repeat=10):
    """Return (output, min wall seconds per run with device-resident inputs)."""
    import time

    x = np.asarray(x, dtype=np.float32)
    W = np.asarray(W, dtype=np.float32)
    runner = _get_runner()
    runner.place_inputs(
        _prepare_in_maps(
            x, W,
            np.asarray(dense_w, dtype=np.float32),
            np.asarray(dense_b, dtype=np.float32),
        )
    )
    out_arrs = runner.run()  # warmup/compile
    times = []
    for _ in range(repeat):
        t0 = time.perf_counter()
        out_arrs = runner.run()
        times.append(time.perf_counter() - t0)
    return _gather_output(runner.results(out_arrs)), min(times)


if __name__ == "__main__":
    nc = _get_nc()
    print("built ok")
